# revision 33
# baseline (speedup 1.0000x reference)
"""Trainium2 Bass kernel for nn_Prior: per-sample MVN log-prob.

Structure exploited: e in {0,1} and (y,e) in {0..3}, so there are only
2 distinct causal and 4 distinct spurious Gaussians.  The covariance is
block-diagonal (two 64x64 blocks), so the per-sample 128x128 Cholesky
collapses to 6 precomputed 64x64 factorizations (tiny host-side tables,
computed from the replicated MLP params in float64).

The batch is sorted by (e, y) on the host and padded to 72 homogeneous
128-sample chunks (9 per core, pure data parallel over 8 cores), so each
chunk needs exactly one causal + one spurious whitening table and no
masking:  S = [W_c (z_c - mu_c) | W_s (z_s - mu_s)]  via two matmuls
(features+ones stationary, per-chunk tables moving),
logp = -0.5 * ||S||^2 + bias.  Squares run on the Activation engine,
row-sums + scale/bias on DVE, three chunks batched per PSUM bank.
"""

import numpy as np
from contextlib import ExitStack

import concourse.bass as bass
import concourse.bacc as bacc
import concourse.tile as tile
import concourse.mybir as mybir
from concourse.bass_utils import run_bass_kernel_spmd

N_CORES = 8
B = 8192
CC = 9                     # chunks per core (72 total, padded)
BC = CC * 128              # padded samples per core (1152)
BP = N_CORES * BC          # padded batch (9216)
Z = 64
R = 8
LOG2PI = float(np.log(2.0 * np.pi))
DT = mybir.dt.float32

_NC_CACHE = {}


def _mlp(ps, x):
    for W, b in ps[:-1]:
        x = np.maximum(x @ W + b, 0.0)
    W, b = ps[-1]
    return x @ W + b


def _softplus(x):
    return np.log1p(np.exp(-np.abs(x))) + np.maximum(x, 0.0)


def _variant_tables(mu, lr, dg):
    """Per-variant whitening matrix W = L^-1 and logdet of cov."""
    n = mu.shape[0]
    Ws, lds = [], []
    for v in range(n):
        cov = lr[v] @ lr[v].T + np.diag(_softplus(dg[v]))
        L = np.linalg.cholesky(cov)
        lds.append(2.0 * np.sum(np.log(np.diag(L))))
        Ws.append(np.linalg.inv(L))
    return np.stack(Ws), np.array(lds)


def _build_nc():
    nc = bacc.Bacc("TRN2", target_bir_lowering=False, debug=False)
    # Per-group (3 chunks = 384 cols) tensors so each processing group only
    # waits on its own DMAs, spread across the SP and Pool queues in usage
    # order.
    G = 384
    wt_d = [nc.declare_dram_parameter(f"wt{g}", [Z + 1, G], DT, isOutput=False)
            for g in range(3)]
    ztc_d = [nc.declare_dram_parameter(f"ztc{g}", [Z + 1, G], DT, isOutput=False)
             for g in range(3)]
    zts_d = [nc.declare_dram_parameter(f"zts{g}", [Z + 1, G], DT, isOutput=False)
             for g in range(3)]
    auxb = nc.declare_dram_parameter("auxb", [128, CC], DT, isOutput=False)
    outp = nc.declare_dram_parameter("out", [BC], DT, isOutput=True)

    with tile.TileContext(nc) as tc, ExitStack() as ctx:
        cpool = ctx.enter_context(tc.tile_pool(name="const", bufs=1))
        wpool = ctx.enter_context(tc.tile_pool(name="work", bufs=3))
        ppool = ctx.enter_context(tc.tile_pool(name="psum", bufs=3, space="PSUM"))

        wt_sb = [cpool.tile([Z + 1, G], DT, tag=f"wt{g}", name=f"wt_sb{g}")
                 for g in range(3)]
        ztc_sb = [cpool.tile([Z + 1, G], DT, tag=f"ztc{g}", name=f"ztc_sb{g}")
                  for g in range(3)]
        zts_sb = [cpool.tile([Z + 1, G], DT, tag=f"zts{g}", name=f"zts_sb{g}")
                  for g in range(3)]
        auxb_sb = cpool.tile([128, CC], DT, tag="auxb")
        def dma_group(g):
            nc.sync.dma_start(wt_sb[g][:], wt_d[g][:])
            nc.gpsimd.dma_start(zts_sb[g][:], zts_d[g][:])
            nc.sync.dma_start(ztc_sb[g][:], ztc_d[g][:])

        dma_group(0)
        nc.gpsimd.dma_start(auxb_sb[:], auxb[:])

        res = cpool.tile([128, 32], DT, tag="res")
        nc.vector.memset(res[:], 0.0)

        for grp in range(3):
            if grp < 2:
                dma_group(grp + 1)
            s_ps = ppool.tile([128, 384], DT, tag="S")
            for t in range(3):
                j = grp * 3 + t
                o = t * 128
                nc.tensor.matmul(s_ps[:, o:o + 64],
                                 ztc_sb[grp][:, o:o + 128],
                                 wt_sb[grp][:, o:o + 64],
                                 start=True, stop=True)
                nc.tensor.matmul(s_ps[:, o + 64:o + 128],
                                 zts_sb[grp][:, o:o + 128],
                                 wt_sb[grp][:, o + 64:o + 128],
                                 start=True, stop=True)
            # ACT: squares for all three chunks in one op (PSUM -> SBUF)
            sq = wpool.tile([128, 384], DT, tag="sq")
            nc.scalar.square(sq[:], s_ps[:])
            # DVE: per-chunk sums ||S||^2 -> [128, 3]
            t3 = wpool.tile([128, 4], DT, tag="t3")
            nc.vector.tensor_reduce(
                out=t3[:, 0:3], in_=sq[:].rearrange("p (t k) -> p t k", t=3),
                axis=mybir.AxisListType.X, op=mybir.AluOpType.add)
            for t in range(3):
                j = grp * 3 + t
                nc.vector.tensor_scalar(
                    out=res[:, j:j + 1], in0=t3[:, t:t + 1],
                    scalar1=-0.5, scalar2=auxb_sb[:, j:j + 1],
                    op0=mybir.AluOpType.mult, op1=mybir.AluOpType.add)

        # res [128, CC] -> out[j*128 + p] : transpose 32x32 blocks via DVE
        tr = cpool.tile([32, 128], DT, tag="tr")
        for i in range(4):
            nc.vector.transpose(tr[0:32, i * 32:(i + 1) * 32],
                                res[i * 32:(i + 1) * 32, 0:32])
        nc.gpsimd.dma_start(outp[:].rearrange("(j p) -> j p", p=128),
                            tr[0:CC, :])
    nc.finalize()
    return nc


def _get_nc():
    if "nc" not in _NC_CACHE:
        _NC_CACHE["nc"] = _build_nc()
    return _NC_CACHE["nc"]


def _prepare(y, e, z, params):
    y = np.asarray(y).astype(np.int64)
    e = np.asarray(e).astype(np.int64)
    z = np.asarray(z).astype(np.float32)
    p64 = {k: [(np.asarray(W, np.float64), np.asarray(b, np.float64))
               for (W, b) in v] for k, v in params.items()}

    # 6 distinct variants: 2 causal (cond on e), 4 spurious (cond on (y,e))
    xc = np.eye(2)
    mu_c = _mlp(p64["mu_c"], xc)
    lr_c = _mlp(p64["lr_c"], xc).reshape(2, Z, R)
    dg_c = _mlp(p64["diag_c"], xc)
    xs = np.zeros((4, 4))
    for yv in range(2):
        for ev in range(2):
            s = 2 * yv + ev
            xs[s, yv] = 1.0
            xs[s, 2 + ev] = 1.0
    mu_s = _mlp(p64["mu_s"], xs)
    lr_s = _mlp(p64["lr_s"], xs).reshape(4, Z, R)
    dg_s = _mlp(p64["diag_s"], xs)

    w_c, ld_c = _variant_tables(mu_c, lr_c, dg_c)
    w_s, ld_s = _variant_tables(mu_s, lr_s, dg_s)

    def wcol(Wm, mu):
        w0 = Wm @ mu
        return np.concatenate([Wm.T, -w0[None, :]], axis=0)  # [65, 64]

    tab_c = [wcol(w_c[v], mu_c[v]) for v in range(2)]
    tab_s = [wcol(w_s[v], mu_s[v]) for v in range(4)]

    # Sort by (e, y): causal groups (by e) and spurious groups (by (y,e))
    # are both contiguous.  Pad each group to a multiple of 128 with
    # duplicated rows, then pad to 72 chunks total.
    g = e * 2 + y
    order = np.argsort(g, kind="stable")
    gs = g[order]
    idx_chunks, chunk_g = [], []
    for gv in range(4):
        ig = order[gs == gv]
        if len(ig) == 0:
            continue
        npad = (-len(ig)) % 128
        ig = np.concatenate([ig, np.repeat(ig[-1:], npad)])
        idx_chunks.append(ig)
        chunk_g += [gv] * (len(ig) // 128)
    n_extra = N_CORES * CC - len(chunk_g)
    assert n_extra >= 0
    if n_extra:
        last = idx_chunks[-1][-1:]
        idx_chunks.append(np.repeat(last, 128 * n_extra))
        chunk_g += [chunk_g[-1]] * n_extra
    perm = np.concatenate(idx_chunks)          # [BP]
    assert perm.shape[0] == BP

    s_idx = 2 * y + e
    bias = (-0.5 * (2 * Z * LOG2PI + ld_c[e] + ld_s[s_idx])).astype(np.float32)

    zp = z[perm]                               # [BP, 128]
    ones = np.ones((1, BP), np.float32)
    ztc = np.concatenate([zp[:, :Z].T, ones], axis=0).astype(np.float32)
    zts = np.concatenate([zp[:, Z:].T, ones], axis=0).astype(np.float32)
    bias_p = bias[perm]                        # [BP]

    # per-chunk fused tables [65, 128] = [W_c(e) | W_s(2y+e)]
    wt = np.empty((Z + 1, N_CORES * CC * 128), np.float32)
    for j, gv in enumerate(chunk_g):
        ev, yv = gv >> 1, gv & 1
        wt[:, j * 128:j * 128 + 64] = tab_c[ev]
        wt[:, j * 128 + 64:(j + 1) * 128] = tab_s[2 * yv + ev]

    G = 384
    in_maps = []
    for c in range(N_CORES):
        sl = slice(c * BC, (c + 1) * BC)
        wt_c_ = wt[:, c * BC:(c + 1) * BC]
        ztc_c = ztc[:, sl]
        zts_c = zts[:, sl]
        m = {"auxb": np.ascontiguousarray(bias_p[sl].reshape(CC, 128).T)}
        for g in range(3):
            gs = slice(g * G, (g + 1) * G)
            m[f"wt{g}"] = np.ascontiguousarray(wt_c_[:, gs])
            m[f"ztc{g}"] = np.ascontiguousarray(ztc_c[:, gs])
            m[f"zts{g}"] = np.ascontiguousarray(zts_c[:, gs])
        in_maps.append(m)
    return in_maps, perm


def run(y, e, z, params, **spmd_kwargs):
    in_maps, perm = _prepare(y, e, z, params)
    nc = _get_nc()
    res = run_bass_kernel_spmd(nc, in_maps, list(range(N_CORES)), **spmd_kwargs)
    out_p = np.concatenate([np.asarray(res.results[c]["out"]).reshape(BC)
                            for c in range(N_CORES)])
    out = np.empty(B, np.float32)
    out[perm] = out_p                          # duplicate pads write equal values
    return out, res


def kernel(y, e, z, params):
    out, _ = run(y, e, z, params)
    return out
